# revision 5
# baseline (speedup 1.0000x reference)
"""Trainium2 Bass kernel for nn_DenseDCLLlayer (DCLL dense spiking layer).

Computation per batch row b:
    isyn = 0.85*prev_isyn + x @ W.T
    vmem = 0.9*prev_vmem + isyn
    eps0 = 0.85*prev_eps0 + x
    eps1 = 0.9*prev_eps1 + eps0
    pv   = eps1 @ W.T
    output = (vmem > 0.5).f32
    pvoutput = log_softmax(sigmoid(pv @ Wo.T + bo), axis=1)

Sharding: data-parallel over batch across 8 NeuronCores (512 rows each),
W / Wo / bo replicated.

Per-core structure:
  Phase 1: eps chain in natural layout; x and eps1 transposed on PE
           (128x128 tiles via identity matmul) into resident SBUF tensors
           xT/eps1T [k=4096, b=512] in float32r.
  Phase 2: per o-slice of 256: W rows loaded naturally, PE-transposed into
           WT [k, o_sl]; gemm1 (stationary=xT tile, moving=WT, N=256)
           accumulates isyn psum in natural [b, o] layout; gemm2
           (stationary=WT chunk, moving=eps1T, N=512) produces pv
           transposed [o, b], feeding the classifier head matmul
           (stationary=WoT chunk, moving=pvT) accumulated over o.
  Phase 3: head finalize: sigmoid(+bo) on [10, 512], PE-transpose to
           [512, 10], log-softmax along free dim.

float32r (FP22) matmuls run at 1 cycle/row for moving dim >= 256; inputs
are rounded to FP22 by the PSUM->SBUF copies that follow each transpose.
"""

from contextlib import ExitStack

import numpy as np

P = 128
ALPHA = 0.9
ALPHAS = 0.85
THRESH = 0.5

B_FULL, D_IN, D_OUT, D_CLS = 4096, 4096, 4096, 10
N_CORES = 8


def build_nc(BS, DIN, DOUT, DCLS, repeat=1, num_devices=N_CORES, osl=256):
    import concourse.bass as bass
    import concourse.mybir as mybir
    import concourse.tile as tile
    from concourse import bacc
    from concourse.masks import make_identity

    f32 = mybir.dt.float32
    f32r = mybir.dt.float32r
    Alu = mybir.AluOpType
    Act = mybir.ActivationFunctionType

    KB = DIN // P            # number of k tiles
    BT = BS // P             # number of b tiles
    OSL = min(osl, DOUT)     # o-slice width (gemm1 moving free dim)
    NOS = DOUT // OSL        # number of o slices
    OC2 = OSL // P           # 128-row o-chunks per slice
    KCH = min(512, DIN)      # k chunk width for staging loads
    NKC = DIN // KCH         # k chunks
    TPK = KCH // P           # transposes per k chunk

    assert DOUT % OSL == 0 and DIN % KCH == 0 and BS % P == 0

    nc = bacc.Bacc(
        "TRN2",
        target_bir_lowering=False,
        debug=False,
        num_devices=num_devices,
        enable_partition_id=False,
    )

    x_d = nc.dram_tensor("x", [BS, DIN], f32, kind="ExternalInput")
    pisyn_d = nc.dram_tensor("prev_isyn", [BS, DOUT], f32, kind="ExternalInput")
    pvmem_d = nc.dram_tensor("prev_vmem", [BS, DOUT], f32, kind="ExternalInput")
    peps0_d = nc.dram_tensor("prev_eps0", [BS, DIN], f32, kind="ExternalInput")
    peps1_d = nc.dram_tensor("prev_eps1", [BS, DIN], f32, kind="ExternalInput")
    w_d = nc.dram_tensor("W", [DOUT, DIN], f32, kind="ExternalInput")
    wo_d = nc.dram_tensor("Wo", [DCLS, DOUT], f32, kind="ExternalInput")
    bo_d = nc.dram_tensor("bo", [1, DCLS], f32, kind="ExternalInput")

    isyn_d = nc.dram_tensor("isyn", [BS, DOUT], f32, kind="ExternalOutput")
    vmem_d = nc.dram_tensor("vmem", [BS, DOUT], f32, kind="ExternalOutput")
    eps0_d = nc.dram_tensor("eps0", [BS, DIN], f32, kind="ExternalOutput")
    eps1_d = nc.dram_tensor("eps1", [BS, DIN], f32, kind="ExternalOutput")
    out_d = nc.dram_tensor("output", [BS, DOUT], f32, kind="ExternalOutput")
    pvout_d = nc.dram_tensor("pvoutput", [BS, DCLS], f32, kind="ExternalOutput")

    with tile.TileContext(nc) as tc:
        for _rep in range(repeat):
            with ExitStack() as ctx:
                _emit_body(
                    ctx, tc, nc, bass, mybir, tile, make_identity,
                    f32, f32r, Alu, Act,
                    BS, DIN, DOUT, DCLS, KB, BT, OSL, NOS, OC2, KCH, NKC, TPK,
                    x_d, pisyn_d, pvmem_d, peps0_d, peps1_d, w_d, wo_d, bo_d,
                    isyn_d, vmem_d, eps0_d, eps1_d, out_d, pvout_d,
                )

    nc.compile()
    return nc


def _emit_body(
    ctx, tc, nc, bass, mybir, tile, make_identity,
    f32, f32r, Alu, Act,
    BS, DIN, DOUT, DCLS, KB, BT, OSL, NOS, OC2, KCH, NKC, TPK,
    x_d, pisyn_d, pvmem_d, peps0_d, peps1_d, w_d, wo_d, bo_d,
    isyn_d, vmem_d, eps0_d, eps1_d, out_d, pvout_d,
):
    ts = bass.ts

    # --- pools ---
    const_p = ctx.enter_context(tc.tile_pool(name="const", bufs=1))
    big_p = ctx.enter_context(tc.tile_pool(name="big", bufs=1))
    wstage_p = ctx.enter_context(tc.tile_pool(name="wstage", bufs=3))
    xstage_p = ctx.enter_context(tc.tile_pool(name="xstage", bufs=2))
    eps_p = ctx.enter_context(tc.tile_pool(name="eps", bufs=2))
    io_p = ctx.enter_context(tc.tile_pool(name="io", bufs=2))
    pvt_p = ctx.enter_context(tc.tile_pool(name="pvt", bufs=2))
    head_p = ctx.enter_context(tc.tile_pool(name="head", bufs=1))

    ps_g1 = ctx.enter_context(tc.tile_pool(name="ps_g1", bufs=2, space="PSUM"))
    ps_g2 = ctx.enter_context(tc.tile_pool(name="ps_g2", bufs=2, space="PSUM"))
    ps_tr = ctx.enter_context(tc.tile_pool(name="ps_tr", bufs=2, space="PSUM"))
    ps_hd = ctx.enter_context(tc.tile_pool(name="ps_hd", bufs=1, space="PSUM"))

    # --- constants ---
    ident = const_p.tile([P, P], f32)
    make_identity(nc, ident[:])

    # bo as per-partition bias [DCLS, 1]
    boT = const_p.tile([DCLS, 1], f32)
    nc.sync.dma_start(boT[:], bo_d[:].rearrange("a c -> c a"))

    # WoT [o, c] tiles: [P, KO, DCLS] where KO = DOUT//P o-chunks.
    # Wo is staged through the w_stage pool in [DCLS, KCH] chunks.
    KO = DOUT // P
    CPW = KCH // P  # o-chunks per Wo stage chunk
    woT = const_p.tile([P, KO, DCLS], f32r)
    for wc in range(DOUT // KCH):
        wo_stage = wstage_p.tile([DCLS, KCH], f32, tag="w_stage")
        nc.sync.dma_start(wo_stage[:], wo_d[:, ts(wc, KCH)])
        ptile = ps_tr.tile([P, 512], f32)
        for i in range(CPW):
            nc.tensor.transpose(
                ptile[:, i * DCLS : (i + 1) * DCLS],
                wo_stage[:, ts(i, P)],
                ident[:DCLS, :DCLS],
            )
        nc.vector.tensor_copy(
            woT[:, wc * CPW : (wc + 1) * CPW, :],
            ptile[:, : CPW * DCLS].rearrange("p (a c) -> p a c", c=DCLS),
        )

    # --- resident transposed operands ---
    xT = big_p.tile([P, KB, BS], f32r)
    eps1T = big_p.tile([P, KB, BS], f32r)

    # --- phase 1: eps chain + x/eps1 transposes ---
    for bt in range(BT):
        for kc in range(NKC):
            x_c = xstage_p.tile([P, KCH], f32, tag="x_c")
            pe0_c = xstage_p.tile([P, KCH], f32, tag="pe0_c")
            pe1_c = xstage_p.tile([P, KCH], f32, tag="pe1_c")
            nc.sync.dma_start(x_c[:], x_d[ts(bt, P), ts(kc, KCH)])
            nc.sync.dma_start(pe0_c[:], peps0_d[ts(bt, P), ts(kc, KCH)])
            nc.sync.dma_start(pe1_c[:], peps1_d[ts(bt, P), ts(kc, KCH)])

            eps0_c = eps_p.tile([P, KCH], f32, tag="eps0_c")
            eps1_c = eps_p.tile([P, KCH], f32, tag="eps1_c")
            # eps0 = 0.85*prev_eps0 + x ; eps1 = 0.9*prev_eps1 + eps0
            nc.vector.scalar_tensor_tensor(
                eps0_c[:], pe0_c[:], ALPHAS, x_c[:], op0=Alu.mult, op1=Alu.add
            )
            nc.vector.scalar_tensor_tensor(
                eps1_c[:], pe1_c[:], ALPHA, eps0_c[:], op0=Alu.mult, op1=Alu.add
            )
            nc.sync.dma_start(eps0_d[ts(bt, P), ts(kc, KCH)], eps0_c[:])
            nc.sync.dma_start(eps1_d[ts(bt, P), ts(kc, KCH)], eps1_c[:])

            # transposes: x_c and eps1_c -> xT/eps1T[:, kc*TPK + j, bt*P:...]
            for src, dstT in ((x_c, xT), (eps1_c, eps1T)):
                nb = 0
                while nb < TPK:
                    nbatch = min(4, TPK - nb)
                    ptile = ps_tr.tile([P, 512], f32)
                    for i in range(nbatch):
                        nc.tensor.transpose(
                            ptile[:, ts(i, P)],
                            src[:, ts(nb + i, P)],
                            ident[:],
                        )
                    for i in range(nbatch):
                        nc.vector.tensor_copy(
                            dstT[:, kc * TPK + nb + i, ts(bt, P)],
                            ptile[:, ts(i, P)],
                        )
                    nb += nbatch

    # --- head accumulator ---
    ps_head = ps_hd.tile([DCLS, BS], f32)

    # --- phase 2: per o-slice ---
    wT = big_p.tile([P, KB, OSL], f32r)
    n_oc = DOUT // P  # total 128-wide o chunks (head accumulation length)

    for osl_i in range(NOS):
        # W transposes, kc-major so low-k WT slots complete first
        for kc in range(NKC):
            for oc2 in range(OC2):
                w_stage = wstage_p.tile([P, KCH], f32, tag="w_stage")
                nc.sync.dma_start(
                    w_stage[:],
                    w_d[osl_i * OSL + oc2 * P : osl_i * OSL + (oc2 + 1) * P,
                        ts(kc, KCH)],
                )
                nb = 0
                while nb < TPK:
                    nbatch = min(4, TPK - nb)
                    ptile = ps_tr.tile([P, 512], f32)
                    for i in range(nbatch):
                        nc.tensor.transpose(
                            ptile[:, ts(i, P)], w_stage[:, ts(nb + i, P)], ident[:]
                        )
                    for i in range(nbatch):
                        nc.vector.tensor_copy(
                            wT[:, kc * TPK + nb + i, ts(oc2, P)],
                            ptile[:, ts(i, P)],
                        )
                    nb += nbatch

        # gemm1: psum[b=128, o=OSL] += xT_tile.T @ WT
        for bt in range(BT):
            psum1 = ps_g1.tile([P, OSL], f32)
            for kt in range(KB):
                nc.tensor.matmul(
                    psum1[:],
                    xT[:, kt, ts(bt, P)],
                    wT[:, kt, :],
                    start=(kt == 0),
                    stop=(kt == KB - 1),
                )
            pis_t = io_p.tile([P, OSL], f32, tag="pis")
            pvm_t = io_p.tile([P, OSL], f32, tag="pvm")
            nc.sync.dma_start(pis_t[:], pisyn_d[ts(bt, P), ts(osl_i, OSL)])
            nc.sync.dma_start(pvm_t[:], pvmem_d[ts(bt, P), ts(osl_i, OSL)])
            isyn_t = io_p.tile([P, OSL], f32, tag="isyn")
            vmem_t = io_p.tile([P, OSL], f32, tag="vmem")
            spk_t = io_p.tile([P, OSL], f32, tag="spk")
            nc.vector.scalar_tensor_tensor(
                isyn_t[:], pis_t[:], ALPHAS, psum1[:], op0=Alu.mult, op1=Alu.add
            )
            nc.vector.scalar_tensor_tensor(
                vmem_t[:], pvm_t[:], ALPHA, isyn_t[:], op0=Alu.mult, op1=Alu.add
            )
            nc.vector.tensor_scalar(
                spk_t[:], vmem_t[:], THRESH, None, op0=Alu.is_gt
            )
            nc.sync.dma_start(isyn_d[ts(bt, P), ts(osl_i, OSL)], isyn_t[:])
            nc.sync.dma_start(vmem_d[ts(bt, P), ts(osl_i, OSL)], vmem_t[:])
            nc.sync.dma_start(out_d[ts(bt, P), ts(osl_i, OSL)], spk_t[:])

        # gemm2: pvT chunk [o=128, b=BS] = sum_k WT_chunk.T @ eps1T
        for oc2 in range(OC2):
            oc = osl_i * OC2 + oc2
            psum2 = ps_g2.tile([P, BS], f32)
            for kt in range(KB):
                nc.tensor.matmul(
                    psum2[:],
                    wT[:, kt, ts(oc2, P)],
                    eps1T[:, kt, :],
                    start=(kt == 0),
                    stop=(kt == KB - 1),
                )
            pvT_t = pvt_p.tile([P, BS], f32r, tag="pvT")
            nc.vector.tensor_copy(pvT_t[:], psum2[:])
            # head: ps_head[c, b] += WoT_chunk.T @ pvT
            nc.tensor.matmul(
                ps_head[:],
                woT[:, oc, :],
                pvT_t[:],
                start=(oc == 0),
                stop=(oc == n_oc - 1),
                skip_group_check=True,
            )

    # --- phase 3: head finalize ---
    # sig[c, b] = sigmoid(ps_head + bo)
    sig = head_p.tile([DCLS, BS], f32)
    nc.scalar.activation(sig[:], ps_head[:], Act.Sigmoid, bias=boT[:], scale=1.0)

    # transpose to sT [b=128, j, c]
    NBJ = BS // P
    sT = head_p.tile([P, NBJ, DCLS], f32)
    ptile = ps_tr.tile([P, 512], f32)
    for j2 in range(NBJ):
        nc.tensor.transpose(
            ptile[:, j2 * DCLS : (j2 + 1) * DCLS],
            sig[:, ts(j2, P)],
            ident[:DCLS, :DCLS],
        )
    nc.vector.tensor_copy(
        sT[:], ptile[:, : NBJ * DCLS].rearrange("p (a c) -> p a c", c=DCLS)
    )

    # log_softmax over c (free dim)
    negm = head_p.tile([P, NBJ], f32)
    nc.vector.tensor_reduce(
        negm[:], sT[:], axis=mybir.AxisListType.X, op=Alu.max, negate=True
    )
    esum = head_p.tile([P, NBJ], f32)
    escr = head_p.tile([P, DCLS], f32)
    for j2 in range(NBJ):
        nc.scalar.activation(
            escr[:], sT[:, j2, :], Act.Exp,
            bias=negm[:, j2 : j2 + 1], scale=1.0,
            accum_out=esum[:, j2 : j2 + 1],
        )
    lse = head_p.tile([P, NBJ], f32)
    nc.scalar.activation(lse[:], esum[:], Act.Ln)
    offs = head_p.tile([P, NBJ], f32)
    nc.vector.tensor_sub(offs[:], negm[:], lse[:])
    pvo = head_p.tile([P, NBJ, DCLS], f32)
    for j2 in range(NBJ):
        nc.vector.tensor_scalar(
            pvo[:, j2, :], sT[:, j2, :], offs[:, j2 : j2 + 1], None, op0=Alu.add
        )
    nc.sync.dma_start(
        pvout_d[:].rearrange("(j p) c -> p j c", p=P), pvo[:]
    )


_NC_CACHE = {}


def _get_nc():
    key = "full"
    if key not in _NC_CACHE:
        _NC_CACHE[key] = build_nc(B_FULL // N_CORES, D_IN, D_OUT, D_CLS)
    return _NC_CACHE[key]


def kernel(**inputs):
    """Takes full unsharded inputs, returns the full outputs tuple."""
    from concourse.bass_utils import run_bass_kernel_spmd

    x = np.ascontiguousarray(inputs["x"], dtype=np.float32)
    prev_isyn = np.ascontiguousarray(inputs["prev_isyn"], dtype=np.float32)
    prev_vmem = np.ascontiguousarray(inputs["prev_vmem"], dtype=np.float32)
    prev_eps0 = np.ascontiguousarray(inputs["prev_eps0"], dtype=np.float32)
    prev_eps1 = np.ascontiguousarray(inputs["prev_eps1"], dtype=np.float32)
    W = np.ascontiguousarray(inputs["W"], dtype=np.float32)
    Wo = np.ascontiguousarray(inputs["Wo"], dtype=np.float32)
    bo = np.ascontiguousarray(inputs["bo"], dtype=np.float32).reshape(1, D_CLS)

    nc = _get_nc()
    BS = B_FULL // N_CORES
    in_maps = []
    for c in range(N_CORES):
        sl = slice(c * BS, (c + 1) * BS)
        in_maps.append({
            "x": x[sl], "prev_isyn": prev_isyn[sl], "prev_vmem": prev_vmem[sl],
            "prev_eps0": prev_eps0[sl], "prev_eps1": prev_eps1[sl],
            "W": W, "Wo": Wo, "bo": bo,
        })

    res = run_bass_kernel_spmd(nc, in_maps, core_ids=list(range(N_CORES)))

    def gather(name):
        return np.concatenate([r[name] for r in res.results], axis=0)

    return (
        gather("isyn"), gather("vmem"), gather("eps0"), gather("eps1"),
        gather("output"), gather("pvoutput"),
    )


# revision 14
# speedup vs baseline: 1.5639x; 1.5639x over previous
"""Trainium2 Bass kernel for nn_DenseDCLLlayer (DCLL dense spiking layer).

Computation per batch row b:
    isyn = 0.85*prev_isyn + x @ W.T
    vmem = 0.9*prev_vmem + isyn
    eps0 = 0.85*prev_eps0 + x
    eps1 = 0.9*prev_eps1 + eps0
    pv   = eps1 @ W.T
    output = (vmem > 0.5).f32
    pvoutput = log_softmax(sigmoid(pv @ Wo.T + bo), axis=1)

Sharding: data-parallel over batch across 8 NeuronCores (512 rows each),
W / Wo / bo replicated.

Per-core structure:
  Phase 1: eps chain in natural layout; x and eps1 transposed on PE
           (128x128 tiles via identity matmul) into resident SBUF tensors
           xT/eps1T [k=4096, b=512] in float32r.
  Phase 2: per o-slice of 256: W rows loaded naturally, PE-transposed into
           WT [k, o_sl]; gemm1 (stationary=xT tile, moving=WT, N=256)
           accumulates isyn psum in natural [b, o] layout; gemm2
           (stationary=WT chunk, moving=eps1T, N=512) produces pv
           transposed [o, b], feeding the classifier head matmul
           (stationary=WoT chunk, moving=pvT) accumulated over o.
  Phase 3: head finalize: sigmoid(+bo) on [10, 512], PE-transpose to
           [512, 10], log-softmax along free dim.

float32r (FP22) matmuls run at 1 cycle/row for moving dim >= 256; inputs
are rounded to FP22 by the PSUM->SBUF copies that follow each transpose.
"""

from contextlib import ExitStack

import numpy as np

P = 128
ALPHA = 0.9
ALPHAS = 0.85
THRESH = 0.5

B_FULL, D_IN, D_OUT, D_CLS = 4096, 4096, 4096, 10
N_CORES = 8


def build_nc(BS, DIN, DOUT, DCLS, repeat=1, num_devices=N_CORES, osl=256):
    import concourse.bass as bass
    import concourse.mybir as mybir
    import concourse.tile as tile
    from concourse import bacc
    from concourse.masks import make_identity

    f32 = mybir.dt.float32
    f32r = mybir.dt.float32r
    Alu = mybir.AluOpType
    Act = mybir.ActivationFunctionType

    KB = DIN // P            # number of k tiles
    BT = BS // P             # number of b tiles
    OSL = min(osl, DOUT)     # o-slice width (gemm1 moving free dim)
    NOS = DOUT // OSL        # number of o slices
    OC2 = OSL // P           # 128-row o-chunks per slice
    KCH = min(1024, DIN)     # k chunk width for staging loads
    NKC = DIN // KCH         # k chunks
    TPK = KCH // P           # transposes per k chunk

    assert DOUT % OSL == 0 and DIN % KCH == 0 and BS % P == 0

    nc = bacc.Bacc(
        "TRN2",
        target_bir_lowering=False,
        debug=False,
        num_devices=num_devices,
        enable_partition_id=False,
    )

    x_d = nc.dram_tensor("x", [BS, DIN], f32, kind="ExternalInput")
    pisyn_d = nc.dram_tensor("prev_isyn", [BS, DOUT], f32, kind="ExternalInput")
    pvmem_d = nc.dram_tensor("prev_vmem", [BS, DOUT], f32, kind="ExternalInput")
    peps0_d = nc.dram_tensor("prev_eps0", [BS, DIN], f32, kind="ExternalInput")
    peps1_d = nc.dram_tensor("prev_eps1", [BS, DIN], f32, kind="ExternalInput")
    w_d = nc.dram_tensor("W", [DOUT, DIN], f32, kind="ExternalInput")
    wo_d = nc.dram_tensor("Wo", [DCLS, DOUT], f32, kind="ExternalInput")
    bo_d = nc.dram_tensor("bo", [1, DCLS], f32, kind="ExternalInput")

    isyn_d = nc.dram_tensor("isyn", [BS, DOUT], f32, kind="ExternalOutput")
    vmem_d = nc.dram_tensor("vmem", [BS, DOUT], f32, kind="ExternalOutput")
    eps0_d = nc.dram_tensor("eps0", [BS, DIN], f32, kind="ExternalOutput")
    eps1_d = nc.dram_tensor("eps1", [BS, DIN], f32, kind="ExternalOutput")
    out_d = nc.dram_tensor("output", [BS, DOUT], f32, kind="ExternalOutput")
    pvout_d = nc.dram_tensor("pvoutput", [BS, DCLS], f32, kind="ExternalOutput")

    with tile.TileContext(nc) as tc:
        for _rep in range(repeat):
            with ExitStack() as ctx:
                _emit_body(
                    ctx, tc, nc, bass, mybir, tile, make_identity,
                    f32, f32r, Alu, Act,
                    BS, DIN, DOUT, DCLS, KB, BT, OSL, NOS, OC2, KCH, NKC, TPK,
                    x_d, pisyn_d, pvmem_d, peps0_d, peps1_d, w_d, wo_d, bo_d,
                    isyn_d, vmem_d, eps0_d, eps1_d, out_d, pvout_d,
                )

    nc.compile()
    return nc


def _emit_body(
    ctx, tc, nc, bass, mybir, tile, make_identity,
    f32, f32r, Alu, Act,
    BS, DIN, DOUT, DCLS, KB, BT, OSL, NOS, OC2, KCH, NKC, TPK,
    x_d, pisyn_d, pvmem_d, peps0_d, peps1_d, w_d, wo_d, bo_d,
    isyn_d, vmem_d, eps0_d, eps1_d, out_d, pvout_d,
):
    ts = bass.ts

    # --- pools ---
    const_p = ctx.enter_context(tc.tile_pool(name="const", bufs=1))
    big_p = ctx.enter_context(tc.tile_pool(name="big", bufs=1))
    wstage_p = ctx.enter_context(tc.tile_pool(name="wstage", bufs=2))
    io_p = ctx.enter_context(tc.tile_pool(name="io", bufs=2))
    pvt_p = ctx.enter_context(tc.tile_pool(name="pvt", bufs=2))
    head_p = ctx.enter_context(tc.tile_pool(name="head", bufs=1))

    ps_g1 = ctx.enter_context(tc.tile_pool(name="ps_g1", bufs=2, space="PSUM"))
    ps_g2 = ctx.enter_context(tc.tile_pool(name="ps_g2", bufs=2, space="PSUM"))
    ps_tr = ctx.enter_context(tc.tile_pool(name="ps_tr", bufs=2, space="PSUM"))
    ps_hd = ctx.enter_context(tc.tile_pool(name="ps_hd", bufs=1, space="PSUM"))

    # --- constants ---
    ident = const_p.tile([P, P], f32)
    make_identity(nc, ident[:])

    # bo as per-partition bias [DCLS, 1]
    boT = const_p.tile([DCLS, 1], f32)
    nc.sync.dma_start(boT[:], bo_d[:].rearrange("a c -> c a"))

    # WoT [o, c] tiles: [P, KO, DCLS] where KO = DOUT//P o-chunks.
    # Wo is staged through the w_stage pool in [DCLS, KCH] chunks.
    KO = DOUT // P
    WCH = min(KCH, DOUT)  # Wo stage chunk width
    CPW = WCH // P  # o-chunks per Wo stage chunk
    woT = const_p.tile([P, KO, DCLS], f32r)
    for wc in range(DOUT // WCH):
        wo_stage = wstage_p.tile([DCLS, WCH], f32, tag="w_stage")
        nc.sync.dma_start(wo_stage[:], wo_d[:, ts(wc, WCH)])
        ptile = ps_tr.tile([P, 512], f32)
        for i in range(CPW):
            nc.tensor.transpose(
                ptile[:, i * DCLS : (i + 1) * DCLS],
                wo_stage[:, ts(i, P)],
                ident[:DCLS, :DCLS],
            )
        nc.vector.tensor_copy(
            woT[:, wc * CPW : (wc + 1) * CPW, :],
            ptile[:, : CPW * DCLS].rearrange("p (a c) -> p a c", c=DCLS),
        )

    # --- resident transposed operands ---
    xT = big_p.tile([P, KB, BS], f32r)
    eps1T = big_p.tile([P, KB, BS], f32r)

    # --- phase 1: eps chain + x/eps1 transposes ---
    # staging pools are scoped to phase 1 so the wT pool can reuse the space
    phase1_ctx = ExitStack()
    xstage_p = phase1_ctx.enter_context(tc.tile_pool(name="xstage", bufs=2))
    eps_p = xstage_p
    for bt in range(BT):
        for kc in range(NKC):
            x_c = xstage_p.tile([P, KCH], f32, tag="x_c")
            pe0_c = xstage_p.tile([P, KCH], f32, tag="pe0_c")
            pe1_c = xstage_p.tile([P, KCH], f32, tag="pe1_c")
            nc.sync.dma_start(x_c[:], x_d[ts(bt, P), ts(kc, KCH)])
            nc.sync.dma_start(pe0_c[:], peps0_d[ts(bt, P), ts(kc, KCH)])
            nc.sync.dma_start(pe1_c[:], peps1_d[ts(bt, P), ts(kc, KCH)])

            eps0_c = eps_p.tile([P, KCH], f32, tag="eps0_c")
            eps1_c = eps_p.tile([P, KCH], f32, tag="eps1_c")
            # eps0 = 0.85*prev_eps0 + x ; eps1 = 0.9*prev_eps1 + eps0
            nc.vector.scalar_tensor_tensor(
                eps0_c[:], pe0_c[:], ALPHAS, x_c[:], op0=Alu.mult, op1=Alu.add
            )
            nc.vector.scalar_tensor_tensor(
                eps1_c[:], pe1_c[:], ALPHA, eps0_c[:], op0=Alu.mult, op1=Alu.add
            )
            nc.sync.dma_start(eps0_d[ts(bt, P), ts(kc, KCH)], eps0_c[:])
            nc.sync.dma_start(eps1_d[ts(bt, P), ts(kc, KCH)], eps1_c[:])

            # transposes: x_c and eps1_c -> xT/eps1T[:, kc*TPK + j, bt*P:...]
            for src, dstT in ((x_c, xT), (eps1_c, eps1T)):
                nb = 0
                while nb < TPK:
                    nbatch = min(4, TPK - nb)
                    ptile = ps_tr.tile([P, 512], f32)
                    for i in range(nbatch):
                        nc.tensor.transpose(
                            ptile[:, ts(i, P)],
                            src[:, ts(nb + i, P)],
                            ident[:],
                        )
                    for i in range(nbatch):
                        nc.vector.tensor_copy(
                            dstT[:, kc * TPK + nb + i, ts(bt, P)],
                            ptile[:, ts(i, P)],
                        )
                    nb += nbatch

    phase1_ctx.close()

    # --- head accumulator ---
    ps_head = ps_hd.tile([DCLS, BS], f32)

    # --- phase 2: per o-slice ---
    # WT is held as rotating k-segments of KSEG k-slots each; a segment
    # matches one W stage chunk, so segments complete (and free) in k order.
    KSEG = TPK
    NSEG = KB // KSEG
    wt_p = ctx.enter_context(tc.tile_pool(name="wt", bufs=NSEG + 1))
    n_oc = DOUT // P  # total 128-wide o chunks (head accumulation length)

    for osl_i in range(NOS):
        segs = []
        # W transposes, kc-major so low-k WT segments complete first
        for kc in range(NKC):
            seg = wt_p.tile([P, KSEG, OSL], f32r, tag="wTseg")
            segs.append(seg)
            for oc2 in range(OC2):
                w_stage = wstage_p.tile([P, KCH], f32, tag="w_stage")
                nc.sync.dma_start(
                    w_stage[:],
                    w_d[osl_i * OSL + oc2 * P : osl_i * OSL + (oc2 + 1) * P,
                        ts(kc, KCH)],
                )
                nb = 0
                while nb < TPK:
                    nbatch = min(4, TPK - nb)
                    ptile = ps_tr.tile([P, 512], f32)
                    for i in range(nbatch):
                        nc.tensor.transpose(
                            ptile[:, ts(i, P)], w_stage[:, ts(nb + i, P)], ident[:]
                        )
                    for i in range(nbatch):
                        nc.vector.tensor_copy(
                            seg[:, nb + i, ts(oc2, P)],
                            ptile[:, ts(i, P)],
                        )
                    nb += nbatch

        # gemm1: psum[b=128, o=OSL] += xT_tile.T @ WT
        for bt in range(BT):
            psum1 = ps_g1.tile([P, OSL], f32)
            for kt in range(KB):
                nc.tensor.matmul(
                    psum1[:],
                    xT[:, kt, ts(bt, P)],
                    segs[kt // KSEG][:, kt % KSEG, :],
                    start=(kt == 0),
                    stop=(kt == KB - 1),
                )
            pis_t = io_p.tile([P, OSL], f32, tag="pis")
            pvm_t = io_p.tile([P, OSL], f32, tag="pvm")
            nc.sync.dma_start(pis_t[:], pisyn_d[ts(bt, P), ts(osl_i, OSL)])
            nc.sync.dma_start(pvm_t[:], pvmem_d[ts(bt, P), ts(osl_i, OSL)])
            isyn_t = io_p.tile([P, OSL], f32, tag="isyn")
            vmem_t = io_p.tile([P, OSL], f32, tag="vmem")
            spk_t = io_p.tile([P, OSL], f32, tag="spk")
            nc.vector.scalar_tensor_tensor(
                isyn_t[:], pis_t[:], ALPHAS, psum1[:], op0=Alu.mult, op1=Alu.add
            )
            nc.vector.scalar_tensor_tensor(
                vmem_t[:], pvm_t[:], ALPHA, isyn_t[:], op0=Alu.mult, op1=Alu.add
            )
            nc.vector.tensor_scalar(
                spk_t[:], vmem_t[:], THRESH, None, op0=Alu.is_gt
            )
            nc.sync.dma_start(isyn_d[ts(bt, P), ts(osl_i, OSL)], isyn_t[:])
            nc.sync.dma_start(vmem_d[ts(bt, P), ts(osl_i, OSL)], vmem_t[:])
            nc.sync.dma_start(out_d[ts(bt, P), ts(osl_i, OSL)], spk_t[:])

        # gemm2: pvT chunk [o=128, b=BS] = sum_k WT_chunk.T @ eps1T
        for oc2 in range(OC2):
            oc = osl_i * OC2 + oc2
            psum2 = ps_g2.tile([P, BS], f32)
            for kt in range(KB):
                nc.tensor.matmul(
                    psum2[:],
                    segs[kt // KSEG][:, kt % KSEG, ts(oc2, P)],
                    eps1T[:, kt, :],
                    start=(kt == 0),
                    stop=(kt == KB - 1),
                )
            pvT_t = pvt_p.tile([P, BS], f32r, tag="pvT")
            nc.vector.tensor_copy(pvT_t[:], psum2[:])
            # head: ps_head[c, b] += WoT_chunk.T @ pvT
            nc.tensor.matmul(
                ps_head[:],
                woT[:, oc, :],
                pvT_t[:],
                start=(oc == 0),
                stop=(oc == n_oc - 1),
                skip_group_check=True,
            )

    # --- phase 3: head finalize ---
    # sig[c, b] = sigmoid(ps_head + bo)
    sig = head_p.tile([DCLS, BS], f32)
    nc.scalar.activation(sig[:], ps_head[:], Act.Sigmoid, bias=boT[:], scale=1.0)

    # transpose to sT [b=128, j, c]
    NBJ = BS // P
    sT = head_p.tile([P, NBJ, DCLS], f32)
    ptile = ps_tr.tile([P, 512], f32)
    for j2 in range(NBJ):
        nc.tensor.transpose(
            ptile[:, j2 * DCLS : (j2 + 1) * DCLS],
            sig[:, ts(j2, P)],
            ident[:DCLS, :DCLS],
        )
    nc.vector.tensor_copy(
        sT[:], ptile[:, : NBJ * DCLS].rearrange("p (a c) -> p a c", c=DCLS)
    )

    # log_softmax over c (free dim)
    negm = head_p.tile([P, NBJ], f32)
    nc.vector.tensor_reduce(
        negm[:], sT[:], axis=mybir.AxisListType.X, op=Alu.max, negate=True
    )
    esum = head_p.tile([P, NBJ], f32)
    escr = head_p.tile([P, DCLS], f32)
    for j2 in range(NBJ):
        nc.scalar.activation(
            escr[:], sT[:, j2, :], Act.Exp,
            bias=negm[:, j2 : j2 + 1], scale=1.0,
            accum_out=esum[:, j2 : j2 + 1],
        )
    lse = head_p.tile([P, NBJ], f32)
    nc.scalar.activation(lse[:], esum[:], Act.Ln)
    offs = head_p.tile([P, NBJ], f32)
    nc.vector.tensor_sub(offs[:], negm[:], lse[:])
    pvo = head_p.tile([P, NBJ, DCLS], f32)
    for j2 in range(NBJ):
        nc.vector.tensor_scalar(
            pvo[:, j2, :], sT[:, j2, :], offs[:, j2 : j2 + 1], None, op0=Alu.add
        )
    nc.sync.dma_start(
        pvout_d[:].rearrange("(j p) c -> p j c", p=P), pvo[:]
    )


_NC_CACHE = {}


def _get_nc():
    key = "full"
    if key not in _NC_CACHE:
        _NC_CACHE[key] = build_nc(B_FULL // N_CORES, D_IN, D_OUT, D_CLS)
    return _NC_CACHE[key]


def kernel(**inputs):
    """Takes full unsharded inputs, returns the full outputs tuple."""
    from concourse.bass_utils import run_bass_kernel_spmd

    x = np.ascontiguousarray(inputs["x"], dtype=np.float32)
    prev_isyn = np.ascontiguousarray(inputs["prev_isyn"], dtype=np.float32)
    prev_vmem = np.ascontiguousarray(inputs["prev_vmem"], dtype=np.float32)
    prev_eps0 = np.ascontiguousarray(inputs["prev_eps0"], dtype=np.float32)
    prev_eps1 = np.ascontiguousarray(inputs["prev_eps1"], dtype=np.float32)
    W = np.ascontiguousarray(inputs["W"], dtype=np.float32)
    Wo = np.ascontiguousarray(inputs["Wo"], dtype=np.float32)
    bo = np.ascontiguousarray(inputs["bo"], dtype=np.float32).reshape(1, D_CLS)

    nc = _get_nc()
    BS = B_FULL // N_CORES
    in_maps = []
    for c in range(N_CORES):
        sl = slice(c * BS, (c + 1) * BS)
        in_maps.append({
            "x": x[sl], "prev_isyn": prev_isyn[sl], "prev_vmem": prev_vmem[sl],
            "prev_eps0": prev_eps0[sl], "prev_eps1": prev_eps1[sl],
            "W": W, "Wo": Wo, "bo": bo,
        })

    res = run_bass_kernel_spmd(nc, in_maps, core_ids=list(range(N_CORES)))

    def gather(name):
        return np.concatenate([r[name] for r in res.results], axis=0)

    return (
        gather("isyn"), gather("vmem"), gather("eps0"), gather("eps1"),
        gather("output"), gather("pvoutput"),
    )


# revision 17
# speedup vs baseline: 1.9222x; 1.2291x over previous
"""Trainium2 Bass kernel for nn_DenseDCLLlayer (DCLL dense spiking layer).

Computation per batch row b:
    isyn = 0.85*prev_isyn + x @ W.T
    vmem = 0.9*prev_vmem + isyn
    eps0 = 0.85*prev_eps0 + x
    eps1 = 0.9*prev_eps1 + eps0
    pv   = eps1 @ W.T
    output = (vmem > 0.5).f32
    pvoutput = log_softmax(sigmoid(pv @ Wo.T + bo), axis=1)

Sharding: data-parallel over batch across 8 NeuronCores (512 rows each),
W / Wo / bo replicated.

Per-core structure:
  Phase 1: eps chain in natural layout; x and eps1 transposed on PE
           (128x128 tiles via identity matmul) into resident SBUF tensors
           xT/eps1T [k=4096, b=512] in float32r.
  Phase 2: per o-slice of 256: W rows loaded naturally, PE-transposed into
           WT [k, o_sl]; gemm1 (stationary=xT tile, moving=WT, N=256)
           accumulates isyn psum in natural [b, o] layout; gemm2
           (stationary=WT chunk, moving=eps1T, N=512) produces pv
           transposed [o, b], feeding the classifier head matmul
           (stationary=WoT chunk, moving=pvT) accumulated over o.
  Phase 3: head finalize: sigmoid(+bo) on [10, 512], PE-transpose to
           [512, 10], log-softmax along free dim.

float32r (FP22) matmuls run at 1 cycle/row for moving dim >= 256; inputs
are rounded to FP22 by the PSUM->SBUF copies that follow each transpose.
"""

from contextlib import ExitStack

import numpy as np

P = 128
ALPHA = 0.9
ALPHAS = 0.85
THRESH = 0.5

B_FULL, D_IN, D_OUT, D_CLS = 4096, 4096, 4096, 10
N_CORES = 8


def build_nc(BS, DIN, DOUT, DCLS, repeat=1, num_devices=N_CORES, osl=256):
    import concourse.bass as bass
    import concourse.mybir as mybir
    import concourse.tile as tile
    from concourse import bacc
    from concourse.masks import make_identity

    f32 = mybir.dt.float32
    f32r = mybir.dt.float32r
    Alu = mybir.AluOpType
    Act = mybir.ActivationFunctionType

    KB = DIN // P            # number of k tiles
    BT = BS // P             # number of b tiles
    OSL = min(osl, DOUT)     # o-slice width (gemm1 moving free dim)
    NOS = DOUT // OSL        # number of o slices
    OC2 = OSL // P           # 128-row o-chunks per slice
    KCH = min(1024, DIN)     # k chunk width for staging loads
    NKC = DIN // KCH         # k chunks
    TPK = KCH // P           # transposes per k chunk

    assert DOUT % OSL == 0 and DIN % KCH == 0 and BS % P == 0

    nc = bacc.Bacc(
        "TRN2",
        target_bir_lowering=False,
        debug=False,
        num_devices=num_devices,
        enable_partition_id=False,
        dynamic_dma_scratch_size=4096,
    )

    x_d = nc.dram_tensor("x", [BS, DIN], f32, kind="ExternalInput")
    pisyn_d = nc.dram_tensor("prev_isyn", [BS, DOUT], f32, kind="ExternalInput")
    pvmem_d = nc.dram_tensor("prev_vmem", [BS, DOUT], f32, kind="ExternalInput")
    peps0_d = nc.dram_tensor("prev_eps0", [BS, DIN], f32, kind="ExternalInput")
    peps1_d = nc.dram_tensor("prev_eps1", [BS, DIN], f32, kind="ExternalInput")
    w_d = nc.dram_tensor("W", [DOUT, DIN], f32, kind="ExternalInput")
    wo_d = nc.dram_tensor("Wo", [DCLS, DOUT], f32, kind="ExternalInput")
    bo_d = nc.dram_tensor("bo", [1, DCLS], f32, kind="ExternalInput")

    isyn_d = nc.dram_tensor("isyn", [BS, DOUT], f32, kind="ExternalOutput")
    vmem_d = nc.dram_tensor("vmem", [BS, DOUT], f32, kind="ExternalOutput")
    eps0_d = nc.dram_tensor("eps0", [BS, DIN], f32, kind="ExternalOutput")
    eps1_d = nc.dram_tensor("eps1", [BS, DIN], f32, kind="ExternalOutput")
    out_d = nc.dram_tensor("output", [BS, DOUT], f32, kind="ExternalOutput")
    pvout_d = nc.dram_tensor("pvoutput", [BS, DCLS], f32, kind="ExternalOutput")

    with tile.TileContext(nc) as tc:
        for _rep in range(repeat):
            with ExitStack() as ctx:
                _emit_body(
                    ctx, tc, nc, bass, mybir, tile, make_identity,
                    f32, f32r, Alu, Act,
                    BS, DIN, DOUT, DCLS, KB, BT, OSL, NOS, OC2, KCH, NKC, TPK,
                    x_d, pisyn_d, pvmem_d, peps0_d, peps1_d, w_d, wo_d, bo_d,
                    isyn_d, vmem_d, eps0_d, eps1_d, out_d, pvout_d,
                )

    nc.compile()
    return nc


def _emit_body(
    ctx, tc, nc, bass, mybir, tile, make_identity,
    f32, f32r, Alu, Act,
    BS, DIN, DOUT, DCLS, KB, BT, OSL, NOS, OC2, KCH, NKC, TPK,
    x_d, pisyn_d, pvmem_d, peps0_d, peps1_d, w_d, wo_d, bo_d,
    isyn_d, vmem_d, eps0_d, eps1_d, out_d, pvout_d,
):
    ts = bass.ts

    # --- pools ---
    const_p = ctx.enter_context(tc.tile_pool(name="const", bufs=1))
    big_p = ctx.enter_context(tc.tile_pool(name="big", bufs=1))
    wstage_p = ctx.enter_context(tc.tile_pool(name="wstage", bufs=2))
    io_p = ctx.enter_context(tc.tile_pool(name="io", bufs=2))
    pvt_p = ctx.enter_context(tc.tile_pool(name="pvt", bufs=2))
    head_p = ctx.enter_context(tc.tile_pool(name="head", bufs=1))

    ps_g1 = ctx.enter_context(tc.tile_pool(name="ps_g1", bufs=2, space="PSUM"))
    ps_g2 = ctx.enter_context(tc.tile_pool(name="ps_g2", bufs=2, space="PSUM"))
    ps_tr = ctx.enter_context(tc.tile_pool(name="ps_tr", bufs=2, space="PSUM"))
    ps_hd = ctx.enter_context(tc.tile_pool(name="ps_hd", bufs=1, space="PSUM"))

    # --- constants ---
    ident = const_p.tile([P, P], f32)
    make_identity(nc, ident[:])

    # bo as per-partition bias [DCLS, 1]
    boT = const_p.tile([DCLS, 1], f32)
    nc.sync.dma_start(boT[:], bo_d[:].rearrange("a c -> c a"))

    # WoT [o, c] tiles: [P, KO, DCLS] where KO = DOUT//P o-chunks.
    # Wo is staged through the w_stage pool in [DCLS, KCH] chunks.
    KO = DOUT // P
    WCH = min(KCH, DOUT)  # Wo stage chunk width
    CPW = WCH // P  # o-chunks per Wo stage chunk
    woT = const_p.tile([P, KO, DCLS], f32r)
    for wc in range(DOUT // WCH):
        wo_stage = wstage_p.tile([DCLS, WCH], f32, tag="w_stage")
        nc.sync.dma_start(wo_stage[:], wo_d[:, ts(wc, WCH)])
        ptile = ps_tr.tile([P, 512], f32)
        for i in range(CPW):
            nc.tensor.transpose(
                ptile[:, i * DCLS : (i + 1) * DCLS],
                wo_stage[:, ts(i, P)],
                ident[:DCLS, :DCLS],
            )
        nc.vector.tensor_copy(
            woT[:, wc * CPW : (wc + 1) * CPW, :],
            ptile[:, : CPW * DCLS].rearrange("p (a c) -> p a c", c=DCLS),
        )

    # --- resident transposed operands ---
    xT = big_p.tile([P, KB, BS], f32r)
    eps1T = big_p.tile([P, KB, BS], f32r)

    # --- phase 1: eps chain + x/eps1 transposes ---
    # staging pools are scoped to phase 1 so the wT pool can reuse the space
    phase1_ctx = ExitStack()
    xstage_p = phase1_ctx.enter_context(tc.tile_pool(name="xstage", bufs=2))
    eps_p = xstage_p
    for bt in range(BT):
        for kc in range(NKC):
            x_c = xstage_p.tile([P, KCH], f32, tag="x_c")
            pe0_c = xstage_p.tile([P, KCH], f32, tag="pe0_c")
            pe1_c = xstage_p.tile([P, KCH], f32, tag="pe1_c")
            nc.sync.dma_start(x_c[:], x_d[ts(bt, P), ts(kc, KCH)])
            nc.sync.dma_start(pe0_c[:], peps0_d[ts(bt, P), ts(kc, KCH)])
            nc.sync.dma_start(pe1_c[:], peps1_d[ts(bt, P), ts(kc, KCH)])

            eps0_c = eps_p.tile([P, KCH], f32, tag="eps0_c")
            eps1_c = eps_p.tile([P, KCH], f32, tag="eps1_c")
            # eps0 = 0.85*prev_eps0 + x ; eps1 = 0.9*prev_eps1 + eps0
            nc.vector.scalar_tensor_tensor(
                eps0_c[:], pe0_c[:], ALPHAS, x_c[:], op0=Alu.mult, op1=Alu.add
            )
            nc.vector.scalar_tensor_tensor(
                eps1_c[:], pe1_c[:], ALPHA, eps0_c[:], op0=Alu.mult, op1=Alu.add
            )
            nc.sync.dma_start(eps0_d[ts(bt, P), ts(kc, KCH)], eps0_c[:])
            nc.sync.dma_start(eps1_d[ts(bt, P), ts(kc, KCH)], eps1_c[:])

            # transposes: x_c and eps1_c -> xT/eps1T[:, kc*TPK + j, bt*P:...]
            for src, dstT in ((x_c, xT), (eps1_c, eps1T)):
                nb = 0
                while nb < TPK:
                    nbatch = min(4, TPK - nb)
                    ptile = ps_tr.tile([P, 512], f32)
                    for i in range(nbatch):
                        nc.tensor.transpose(
                            ptile[:, ts(i, P)],
                            src[:, ts(nb + i, P)],
                            ident[:],
                        )
                    for i in range(nbatch):
                        nc.vector.tensor_copy(
                            dstT[:, kc * TPK + nb + i, ts(bt, P)],
                            ptile[:, ts(i, P)],
                        )
                    nb += nbatch

    phase1_ctx.close()

    # --- head accumulator ---
    ps_head = ps_hd.tile([DCLS, BS], f32)

    # --- phase 2: per o-slice ---
    # WT is held as rotating k-segments of KSEG k-slots each; a segment
    # matches one W stage chunk, so segments complete (and free) in k order.
    KSEG = TPK
    NSEG = KB // KSEG
    wt_p = ctx.enter_context(tc.tile_pool(name="wt", bufs=NSEG + 1))
    n_oc = DOUT // P  # total 128-wide o chunks (head accumulation length)

    for osl_i in range(NOS):
        segs = []
        # W transposes, kc-major so low-k WT segments complete first
        for kc in range(NKC):
            seg = wt_p.tile([P, KSEG, OSL], f32r, tag="wTseg")
            segs.append(seg)
            for oc2 in range(OC2):
                w_stage = wstage_p.tile([P, KCH], f32, tag="w_stage")
                nc.sync.dma_start(
                    w_stage[:],
                    w_d[osl_i * OSL + oc2 * P : osl_i * OSL + (oc2 + 1) * P,
                        ts(kc, KCH)],
                )
                nb = 0
                while nb < TPK:
                    nbatch = min(4, TPK - nb)
                    ptile = ps_tr.tile([P, 512], f32)
                    for i in range(nbatch):
                        nc.tensor.transpose(
                            ptile[:, ts(i, P)], w_stage[:, ts(nb + i, P)], ident[:]
                        )
                    for i in range(nbatch):
                        nc.vector.tensor_copy(
                            seg[:, nb + i, ts(oc2, P)],
                            ptile[:, ts(i, P)],
                        )
                    nb += nbatch

        # both gemms share the stationary WT chunk; moving operands xT and
        # eps1T cover the full per-core batch (N=BS) in one matmul each.
        # gemm1 result arrives transposed [o=128, b=BS] and is PE-transposed
        # back to natural layout for the leaky-integrator chain.
        z1_nat = {}
        for oc2 in range(OC2):
            oc = osl_i * OC2 + oc2
            psum1 = ps_g1.tile([P, BS], f32)
            psum2 = ps_g2.tile([P, BS], f32)
            for kt in range(KB):
                wchunk = segs[kt // KSEG][:, kt % KSEG, ts(oc2, P)]
                nc.tensor.matmul(
                    psum1[:], wchunk, xT[:, kt, :],
                    start=(kt == 0), stop=(kt == KB - 1),
                )
                nc.tensor.matmul(
                    psum2[:], wchunk, eps1T[:, kt, :],
                    start=(kt == 0), stop=(kt == KB - 1),
                )
            # z1T chunk -> SBUF -> transpose back to natural [b, o] blocks
            z1T_t = pvt_p.tile([P, BS], f32, tag="z1T")
            nc.vector.tensor_copy(z1T_t[:], psum1[:])
            ptile = ps_tr.tile([P, 512], f32)
            for bt in range(BT):
                nc.tensor.transpose(
                    ptile[:, ts(bt, P)], z1T_t[:, ts(bt, P)], ident[:]
                )
            for bt in range(BT):
                if bt not in z1_nat:
                    z1n = io_p.tile([P, OSL], f32, tag=f"z1n{bt}", name=f"z1n{bt}")
                    z1_nat[bt] = z1n
                nc.vector.tensor_copy(
                    z1_nat[bt][:, ts(oc2, P)], ptile[:, ts(bt, P)]
                )

            # pv chunk: -> SBUF (fp32r) + head accumulation
            pvT_t = pvt_p.tile([P, BS], f32r, tag="pvT")
            nc.vector.tensor_copy(pvT_t[:], psum2[:])
            nc.tensor.matmul(
                ps_head[:],
                woT[:, oc, :],
                pvT_t[:],
                start=(oc == 0),
                stop=(oc == n_oc - 1),
                skip_group_check=True,
            )

        # leaky-integrator chain per b-tile on the assembled natural z1
        for bt in range(BT):
            pis_t = io_p.tile([P, OSL], f32, tag="pis")
            pvm_t = io_p.tile([P, OSL], f32, tag="pvm")
            nc.sync.dma_start(pis_t[:], pisyn_d[ts(bt, P), ts(osl_i, OSL)])
            nc.sync.dma_start(pvm_t[:], pvmem_d[ts(bt, P), ts(osl_i, OSL)])
            isyn_t = io_p.tile([P, OSL], f32, tag="isyn")
            vmem_t = io_p.tile([P, OSL], f32, tag="vmem")
            spk_t = io_p.tile([P, OSL], f32, tag="spk")
            nc.vector.scalar_tensor_tensor(
                isyn_t[:], pis_t[:], ALPHAS, z1_nat[bt][:],
                op0=Alu.mult, op1=Alu.add,
            )
            nc.vector.scalar_tensor_tensor(
                vmem_t[:], pvm_t[:], ALPHA, isyn_t[:], op0=Alu.mult, op1=Alu.add
            )
            nc.vector.tensor_scalar(
                spk_t[:], vmem_t[:], THRESH, None, op0=Alu.is_gt
            )
            nc.sync.dma_start(isyn_d[ts(bt, P), ts(osl_i, OSL)], isyn_t[:])
            nc.sync.dma_start(vmem_d[ts(bt, P), ts(osl_i, OSL)], vmem_t[:])
            nc.sync.dma_start(out_d[ts(bt, P), ts(osl_i, OSL)], spk_t[:])

    # --- phase 3: head finalize ---
    # sig[c, b] = sigmoid(ps_head + bo)
    sig = head_p.tile([DCLS, BS], f32)
    nc.scalar.activation(sig[:], ps_head[:], Act.Sigmoid, bias=boT[:], scale=1.0)

    # transpose to sT [b=128, j, c]
    NBJ = BS // P
    sT = head_p.tile([P, NBJ, DCLS], f32)
    ptile = ps_tr.tile([P, 512], f32)
    for j2 in range(NBJ):
        nc.tensor.transpose(
            ptile[:, j2 * DCLS : (j2 + 1) * DCLS],
            sig[:, ts(j2, P)],
            ident[:DCLS, :DCLS],
        )
    nc.vector.tensor_copy(
        sT[:], ptile[:, : NBJ * DCLS].rearrange("p (a c) -> p a c", c=DCLS)
    )

    # log_softmax over c (free dim)
    negm = head_p.tile([P, NBJ], f32)
    nc.vector.tensor_reduce(
        negm[:], sT[:], axis=mybir.AxisListType.X, op=Alu.max, negate=True
    )
    esum = head_p.tile([P, NBJ], f32)
    escr = head_p.tile([P, DCLS], f32)
    for j2 in range(NBJ):
        nc.scalar.activation(
            escr[:], sT[:, j2, :], Act.Exp,
            bias=negm[:, j2 : j2 + 1], scale=1.0,
            accum_out=esum[:, j2 : j2 + 1],
        )
    lse = head_p.tile([P, NBJ], f32)
    nc.scalar.activation(lse[:], esum[:], Act.Ln)
    offs = head_p.tile([P, NBJ], f32)
    nc.vector.tensor_sub(offs[:], negm[:], lse[:])
    pvo = head_p.tile([P, NBJ, DCLS], f32)
    for j2 in range(NBJ):
        nc.vector.tensor_scalar(
            pvo[:, j2, :], sT[:, j2, :], offs[:, j2 : j2 + 1], None, op0=Alu.add
        )
    nc.sync.dma_start(
        pvout_d[:].rearrange("(j p) c -> p j c", p=P), pvo[:]
    )


_NC_CACHE = {}


def _get_nc():
    key = "full"
    if key not in _NC_CACHE:
        _NC_CACHE[key] = build_nc(B_FULL // N_CORES, D_IN, D_OUT, D_CLS)
    return _NC_CACHE[key]


def kernel(**inputs):
    """Takes full unsharded inputs, returns the full outputs tuple."""
    from concourse.bass_utils import run_bass_kernel_spmd

    x = np.ascontiguousarray(inputs["x"], dtype=np.float32)
    prev_isyn = np.ascontiguousarray(inputs["prev_isyn"], dtype=np.float32)
    prev_vmem = np.ascontiguousarray(inputs["prev_vmem"], dtype=np.float32)
    prev_eps0 = np.ascontiguousarray(inputs["prev_eps0"], dtype=np.float32)
    prev_eps1 = np.ascontiguousarray(inputs["prev_eps1"], dtype=np.float32)
    W = np.ascontiguousarray(inputs["W"], dtype=np.float32)
    Wo = np.ascontiguousarray(inputs["Wo"], dtype=np.float32)
    bo = np.ascontiguousarray(inputs["bo"], dtype=np.float32).reshape(1, D_CLS)

    nc = _get_nc()
    BS = B_FULL // N_CORES
    in_maps = []
    for c in range(N_CORES):
        sl = slice(c * BS, (c + 1) * BS)
        in_maps.append({
            "x": x[sl], "prev_isyn": prev_isyn[sl], "prev_vmem": prev_vmem[sl],
            "prev_eps0": prev_eps0[sl], "prev_eps1": prev_eps1[sl],
            "W": W, "Wo": Wo, "bo": bo,
        })

    res = run_bass_kernel_spmd(nc, in_maps, core_ids=list(range(N_CORES)))

    def gather(name):
        return np.concatenate([r[name] for r in res.results], axis=0)

    return (
        gather("isyn"), gather("vmem"), gather("eps0"), gather("eps1"),
        gather("output"), gather("pvoutput"),
    )
